# revision 18
# baseline (speedup 1.0000x reference)
"""Trainium2 Bass kernel for nn_InternalMAFE_59270548684863.

Key facts (hardcoded from the problem):
  - Output depends ONLY on branch 1 (p=7, n=288) of the reference; the
    n2=1008 branch feeds a dead projection and is never computed.
  - out = o1 @ proj_len_w.T + proj_len_b,  o1 = branch(x, 7, h1, w_k1, w_v1, ...)
  - Softmax normalizes over the batch axis, so we batch-shard (512 rows/core)
    and AllReduce the per-(slice, feature) exp-sums (a [128,24] f32 buffer).
    Constant-shift softmax (exp(s*scale - 50)) avoids a cross-core max pass.
  - s = h1 @ (x_i w_k)^T is fused as W_hk = h1 @ w_k^T (one 288^3 product)
    so each slice needs only ONE matmul chain for the logits.
  - All matmuls run in bf16 (fp32 matmul is a 2-pass LOW_HIGH on trn2 PE);
    PSUM accumulation, softmax and the gated scan stay fp32.
  - Schedule: all logit/exp work first -> AllReduce fires mid-kernel and is
    hidden behind the v-matmuls and the proj_len_w de-interleave transposes.
"""

import math

import numpy as np

import concourse.bacc as bacc
import concourse.masks as masks
import concourse.mybir as mybir
import concourse.tile as tile
from concourse.bass_utils import run_bass_kernel_spmd

N_CORES = 8
B = 4096
BL = B // N_CORES  # 512 rows per core
INP = 2016
P1 = 7
N1 = 288
SEQ = 1024
SCALE = 1.0 / math.sqrt(N1)
SHIFT = -50.0
F32 = mybir.dt.float32
BF16 = mybir.dt.bfloat16
CH = [(0, 128), (128, 128), (256, 32)]
AF = mybir.ActivationFunctionType


def build():
    nc = bacc.Bacc(
        "TRN2", target_bir_lowering=False, debug=False, num_devices=N_CORES
    )
    x = nc.dram_tensor("x", [BL, INP], F32, kind="ExternalInput").ap()
    wk = nc.dram_tensor("w_k1", [N1, N1], F32, kind="ExternalInput").ap()
    wv = nc.dram_tensor("w_v1", [N1, N1], F32, kind="ExternalInput").ap()
    h1 = nc.dram_tensor("h1", [N1, N1], F32, kind="ExternalInput").ap()
    a1 = nc.dram_tensor("alpha1", [1], F32, kind="ExternalInput").ap()
    a2 = nc.dram_tensor("alpha2", [1], F32, kind="ExternalInput").ap()
    b1 = nc.dram_tensor("beta1", [1], F32, kind="ExternalInput").ap()
    b2 = nc.dram_tensor("beta2", [1], F32, kind="ExternalInput").ap()
    plw = nc.dram_tensor("proj_len_w", [SEQ, INP], F32, kind="ExternalInput").ap()
    plb = nc.dram_tensor("proj_len_b", [SEQ], F32, kind="ExternalInput").ap()
    out = nc.dram_tensor("out", [BL, SEQ], F32, kind="ExternalOutput").ap()

    with tile.TileContext(nc) as tc:
        with (
            tc.tile_pool(name="const", bufs=1) as cpool,
            tc.tile_pool(name="plwn", bufs=1) as plwpool,
            tc.tile_pool(name="plwb", bufs=4) as plwbpool,
            tc.tile_pool(name="rk", bufs=1) as rkpool,
            tc.tile_pool(name="dram", bufs=1, space="DRAM") as dpool,
        ):
            # ---------------- constants ----------------
            ident = cpool.tile([128, 128], BF16, tag="ident", name="ident")
            masks.make_identity(nc, ident[:])
            ones = cpool.tile([1, 128], BF16, tag="ones", name="ones")
            nc.vector.memset(ones[:], 1.0)

            scal = cpool.tile([1, 4], F32, tag="scal", name="scal")
            for idx, ap in enumerate((a1, a2, b1, b2)):
                nc.sync.dma_start(scal[0:1, idx : idx + 1], ap[:])

            plb_sb = cpool.tile([1, SEQ], BF16, tag="plb", name="plb")
            plb_f = cpool.tile([1, SEQ], F32, tag="plb_f", name="plb_f")
            nc.sync.dma_start(plb_f[:], plb[:])
            nc.vector.tensor_copy(plb_sb[:], plb_f[:])

            densb = cpool.tile([128, 24], F32, tag="densb", name="densb")
            nc.vector.memset(densb[:], 0.0)
            shiftc = cpool.tile([128, 1], F32, tag="shiftc", name="shiftc")
            nc.vector.memset(shiftc[:], SHIFT)
            den_all = cpool.tile([128, 24], F32, tag="den_all", name="den_all")
            recip = cpool.tile([128, 24], F32, tag="recip", name="recip")

            cc_in = dpool.tile([128, 24], F32)
            cc_out = dpool.tile([128, 24], F32, addr_space="Shared")

            # fp32 scan state (bf16 mirrors are allocated in phase C)
            ys = [
                [cpool.tile([cnt, BL], F32, tag=f"ys{i}_{c}", name=f"ys{i}_{c}") for c, (j0, cnt) in enumerate(CH)]
                for i in range(P1)
            ]

            # ---------------- phase A/B: weights, x, logits, AR, vT --------
            with (
                tc.tile_pool(name="xn", bufs=2) as xpool,
                tc.tile_pool(name="xnb", bufs=4) as xbpool,
                tc.tile_pool(name="xiT", bufs=1) as xtpool,
                tc.tile_pool(name="ee", bufs=1) as epool,
                tc.tile_pool(name="psT", bufs=2, space="PSUM") as psT,
                tc.tile_pool(name="psS", bufs=2, space="PSUM") as psS,
                tc.tile_pool(name="psV", bufs=2, space="PSUM") as psV,
            ):
                # broadcast the 4 gate scalars to all 128 partitions via PE
                onesf = cpool.tile([1, 128], F32, tag="onesf", name="onesf")
                nc.vector.memset(onesf[:], 1.0)
                pbc = psS.tile([128, 512], F32, tag="ps_st", name="ps_bc")
                nc.tensor.matmul(pbc[:, 0:4], onesf[:], scal[:], start=True, stop=True)
                bcast = cpool.tile([128, 4], F32, tag="bcast", name="bcast")
                nc.vector.tensor_copy(bcast[:], pbc[:, 0:4])

                # weights -> bf16
                wk_b, wv_b, h1_b = [], [], []
                for t, (m0, mc) in enumerate(CH):
                    wtf = xpool.tile([mc, N1], F32, tag="wtmp", name="wtmp", bufs=3)
                    nc.sync.dma_start(wtf[:], wk[m0 : m0 + mc, :])
                    wt = cpool.tile([mc, N1], BF16, tag=f"wkb{t}", name=f"wkb{t}")
                    nc.vector.tensor_copy(wt[:], wtf[:])
                    wk_b.append(wt)
                    vtf = xpool.tile([mc, N1], F32, tag="wtmp", name="wtmp", bufs=3)
                    nc.sync.dma_start(vtf[:], wv[m0 : m0 + mc, :])
                    vt = cpool.tile([mc, N1], BF16, tag=f"wvb{t}", name=f"wvb{t}")
                    nc.vector.tensor_copy(vt[:], vtf[:])
                    wv_b.append(vt)
                    htf = xpool.tile([mc, N1], F32, tag="wtmp", name="wtmp", bufs=3)
                    nc.sync.dma_start(htf[:], h1[m0 : m0 + mc, :])
                    ht = cpool.tile([mc, N1], BF16, tag=f"h1b{t}", name=f"h1b{t}")
                    nc.vector.tensor_copy(ht[:], htf[:])
                    h1_b.append(ht)

                # h1T[l, j] = h1[j, l] and wkT[l, m] = wk[m, l]  (bf16)
                h1T, wkT = [], []
                for lt, (l0, lc) in enumerate(CH):
                    ps = psT.tile([128, 512], BF16, tag="tp", name="tp")
                    for jt, (j0, jc) in enumerate(CH):
                        nc.tensor.transpose(
                            ps[0:lc, j0 : j0 + jc],
                            h1_b[jt][:, l0 : l0 + lc],
                            ident[0:jc, 0:jc],
                        )
                    hT = cpool.tile([lc, N1], BF16, tag=f"h1T{lt}", name=f"h1T{lt}")
                    nc.vector.tensor_copy(hT[:], ps[0:lc, 0:N1])
                    h1T.append(hT)
                    ps2 = psT.tile([128, 512], BF16, tag="tp", name="tp")
                    for mt, (m0, mc) in enumerate(CH):
                        nc.tensor.transpose(
                            ps2[0:lc, m0 : m0 + mc],
                            wk_b[mt][:, l0 : l0 + lc],
                            ident[0:mc, 0:mc],
                        )
                    wTl = cpool.tile([lc, N1], BF16, tag=f"wkT{lt}", name=f"wkT{lt}")
                    nc.vector.tensor_copy(wTl[:], ps2[0:lc, 0:N1])
                    wkT.append(wTl)

                # W_hkT[m, j] = sum_l wk[m,l] h1[j,l]: lhsT=wkT, rhs=h1T (K=l)
                whkT = []
                for mt, (m0, mc) in enumerate(CH):
                    pw = psS.tile([128, 512], F32, tag="ps_st", name="ps_whk")
                    for lt, (l0, lc) in enumerate(CH):
                        nc.tensor.matmul(
                            pw[0:mc, 0:N1],
                            wkT[lt][:, m0 : m0 + mc],
                            h1T[lt][:],
                            start=(lt == 0),
                            stop=(lt == 2),
                        )
                    wTt = cpool.tile([mc, N1], BF16, tag=f"whkT{mt}", name=f"whkT{mt}")
                    nc.vector.tensor_copy(wTt[:], pw[0:mc, 0:N1])
                    whkT.append(wTt)

                # x shard: fp32 load -> bf16 convert
                xnb = []
                for bt in range(4):
                    xt = xpool.tile([128, INP], F32, tag="xn", name="xn")
                    nc.sync.dma_start(xt[:], x[bt * 128 : (bt + 1) * 128, :])
                    xb = xbpool.tile([128, INP], BF16, tag="xnb", name="xnb")
                    nc.vector.tensor_copy(
                        xb[:].rearrange("p (i j) -> p i j", i=P1),
                        xt[:].rearrange("p (j i) -> p j i", i=P1).rearrange("p j i -> p i j"),
                    )
                    xnb.append(xb)

                # prefetch plw half-0 (DMA + bf16 cast) so its de-interleave
                # transposes are ready to fill the AllReduce window
                pw4_h0 = []
                for st in range(4):
                    pwt = plwpool.tile([128, INP], F32, tag="plwn", name="plwn")
                    nc.sync.dma_start(pwt[:], plw[st * 128 : (st + 1) * 128, :])
                    pwb = plwbpool.tile([128, INP], BF16, tag="plwb", name="plwb")
                    nc.vector.tensor_copy(
                        pwb[:].rearrange("p (i j) -> p i j", i=P1),
                        pwt[:].rearrange("p (j i) -> p j i", i=P1).rearrange("p j i -> p i j"),
                    )
                    pw4_h0.append(pwb)

                # all de-interleaving transposes + all logits/exp first so the
                # AllReduce can fire while vT / plw transposes run
                xiT = [[None] * 3 for _ in range(P1)]
                E = [[None] * 3 for _ in range(P1)]
                for i in range(P1):
                    for c, (j0, cnt) in enumerate(CH):
                        xi = xtpool.tile([cnt, BL], BF16, tag=f"xiT{i}_{c}", name=f"xiT{i}_{c}")
                        if cnt == 128:
                            for bt in range(4):
                                s_ap = xnb[bt][:, i * N1 + j0 : i * N1 + j0 + cnt]
                                nc.sync.dma_start_transpose(
                                    xi[:, bt * 128 : (bt + 1) * 128], s_ap
                                )
                        else:
                            ps = psT.tile([128, 512], BF16, tag="tp", name="tp")
                            for bt in range(4):
                                s_ap = xnb[bt][:, i * N1 + j0 : i * N1 + j0 + cnt]
                                nc.tensor.transpose(
                                    ps[0:cnt, bt * 128 : (bt + 1) * 128],
                                    s_ap,
                                    ident[:],
                                )
                            nc.vector.tensor_copy(xi[:], ps[0:cnt, :])
                        xiT[i][c] = xi

                    for jt, (j0, jc) in enumerate(CH):
                        pst = psS.tile([128, 512], F32, tag="ps_st", name="ps_st")
                        for lt, (l0, lc) in enumerate(CH):
                            nc.tensor.matmul(
                                pst[0:jc, :],
                                whkT[lt][:, j0 : j0 + jc],
                                xiT[i][lt][:],
                                start=(lt == 0),
                                stop=(lt == 2),
                            )
                        ec = epool.tile([jc, BL], F32, tag=f"e{i}_{jt}", name=f"e{i}_{jt}")
                        col = i * 3 + jt
                        nc.scalar.activation(
                            ec[:],
                            pst[0:jc, :],
                            AF.Exp,
                            bias=shiftc[0:jc, 0:1],
                            scale=SCALE,
                            accum_out=densb[0:jc, col : col + 1],
                        )
                        E[i][jt] = ec

                # ---- AllReduce of exp-sums (overlaps vT + plw transposes) --
                nc.gpsimd.dma_start(cc_in[:], densb[:])
                nc.gpsimd.collective_compute(
                    "AllReduce",
                    mybir.AluOpType.add,
                    replica_groups=[list(range(N_CORES))],
                    ins=[cc_in[:]],
                    outs=[cc_out[:]],
                )

                # vT = (x_i @ wv)^T ; ys = vT * E (normalized later)
                for i in range(P1):
                    for ntc, (n0, ncnt) in enumerate(CH):
                        pv = psV.tile([128, 512], F32, tag="ps_vt", name="ps_vt")
                        for mt, (m0, mc) in enumerate(CH):
                            nc.tensor.matmul(
                                pv[0:ncnt, :],
                                wv_b[mt][:, n0 : n0 + ncnt],
                                xiT[i][mt][:],
                                start=(mt == 0),
                                stop=(mt == 2),
                            )
                        nc.vector.tensor_mul(ys[i][ntc][:], pv[0:ncnt, :], E[i][ntc][:])

            nc.gpsimd.dma_start(den_all[:], cc_out[:])
            nc.vector.reciprocal(recip[:], den_all[:])

            # ---------------- phase C: plw K-tiles, scan, projection -------
            with (
                tc.tile_pool(name="ysb", bufs=1) as ysbpool,
                tc.tile_pool(name="tmp", bufs=1) as tmppool,
                tc.tile_pool(name="osb", bufs=2) as outpool,
                tc.tile_pool(name="psT2", bufs=2, space="PSUM") as psT2,
                tc.tile_pool(name="psP", bufs=4, space="PSUM") as psP,
            ):
                ysb = [
                    [ysbpool.tile([cnt, BL], BF16, tag=f"ysb{i}_{c}", name=f"ysb{i}_{c}") for c, (j0, cnt) in enumerate(CH)]
                    for i in range(P1)
                ]
                rk_halves = [[[None] * 3 for _ in range(P1)] for _ in range(2)]
                scan_emitted = False
                for half in range(2):
                    # load 4 plw row-tiles, convert to bf16, de-interleave
                    if half == 0:
                        pw4 = pw4_h0
                    else:
                        pw4 = []
                        for st in range(4):
                            pwt = plwpool.tile([128, INP], F32, tag="plwn", name="plwn")
                            r0 = (half * 4 + st) * 128
                            nc.sync.dma_start(pwt[:], plw[r0 : r0 + 128, :])
                            pwb = plwbpool.tile([128, INP], BF16, tag="plwb", name="plwb")
                            nc.vector.tensor_copy(
                                pwb[:].rearrange("p (i j) -> p i j", i=P1),
                                pwt[:].rearrange("p (j i) -> p j i", i=P1).rearrange("p j i -> p i j"),
                            )
                            pw4.append(pwb)
                    rk = rk_halves[half]
                    for i in range(P1):
                        for c, (j0, cnt) in enumerate(CH):
                            rkt = rkpool.tile([cnt, 512], BF16, tag=f"rk{i}_{c}", name=f"rk{i}_{c}")
                            if cnt == 128:
                                for st in range(4):
                                    s_ap = pw4[st][:, i * N1 + j0 : i * N1 + j0 + cnt]
                                    nc.sync.dma_start_transpose(
                                        rkt[:, st * 128 : (st + 1) * 128], s_ap
                                    )
                            else:
                                ps = psT2.tile([128, 512], BF16, tag="tp2", name="tp2")
                                for st in range(4):
                                    s_ap = pw4[st][:, i * N1 + j0 : i * N1 + j0 + cnt]
                                    nc.tensor.transpose(
                                        ps[0:cnt, st * 128 : (st + 1) * 128],
                                        s_ap,
                                        ident[:],
                                    )
                                nc.vector.tensor_copy(rkt[:], ps[0:cnt, :])
                            rk[i][c] = rkt

                    if not scan_emitted:
                        # normalize + gated scan; bf16 mirrors for projection
                        scan_emitted = True
                        for i in range(P1):
                            for c, (j0, cnt) in enumerate(CH):
                                col = i * 3 + c
                                nc.scalar.mul(
                                    ys[i][c][:],
                                    ys[i][c][:],
                                    mul=recip[0:cnt, col : col + 1],
                                )
                            if i >= 1:
                                for c, (j0, cnt) in enumerate(CH):
                                    tt = tmppool.tile([cnt, BL], F32, tag=f"tt{c}", name=f"tt{c}")
                                    ts = tmppool.tile([cnt, BL], F32, tag=f"ts{c}", name=f"ts{c}")
                                    nc.scalar.activation(
                                        tt[:],
                                        ys[i - 1][c][:],
                                        AF.Tanh,
                                        bias=bcast[0:cnt, 2:3],
                                        scale=bcast[0:cnt, 0:1],
                                    )
                                    nc.scalar.activation(
                                        ts[:],
                                        ys[i - 1][c][:],
                                        AF.Sigmoid,
                                        bias=bcast[0:cnt, 3:4],
                                        scale=bcast[0:cnt, 1:2],
                                    )
                                    nc.vector.tensor_mul(tt[:], tt[:], ts[:])
                                    nc.vector.tensor_add(
                                        ys[i][c][:], ys[i][c][:], tt[:]
                                    )
                            for c, (j0, cnt) in enumerate(CH):
                                if c % 2 == 0:
                                    nc.scalar.copy(ysb[i][c][:], ys[i][c][:])
                                else:
                                    nc.vector.tensor_copy(ysb[i][c][:], ys[i][c][:])

                    # projection for this s-half: 4 batch groups of 128
                    pps = []
                    for bc in range(4):
                        pp = psP.tile([128, 512], F32, tag="pj", name="pj")
                        nc.tensor.matmul(
                            pp[:],
                            ones[:],
                            plb_sb[0:1, half * 512 : (half + 1) * 512],
                            start=True,
                            stop=False,
                        )
                        pps.append(pp)
                    for i in range(P1):
                        for c, (j0, cnt) in enumerate(CH):
                            last = i == P1 - 1 and c == 2
                            for bc in range(4):
                                nc.tensor.matmul(
                                    pps[bc][:],
                                    ysb[i][c][:, bc * 128 : (bc + 1) * 128],
                                    rk[i][c][:],
                                    start=False,
                                    stop=last,
                                )
                    for bc in range(4):
                        ob = outpool.tile([128, 512], F32, tag="osb", name="osb")
                        nc.vector.tensor_copy(ob[:], pps[bc][:])
                        nc.sync.dma_start(
                            out[bc * 128 : (bc + 1) * 128, half * 512 : (half + 1) * 512],
                            ob[:],
                        )

    nc.compile()
    return nc


_NC = None


def _get_nc():
    global _NC
    if _NC is None:
        _NC = build()
    return _NC


def run(inputs, trace=False):
    nc = _get_nc()
    rep_keys = [
        "w_k1",
        "w_v1",
        "h1",
        "alpha1",
        "alpha2",
        "beta1",
        "beta2",
        "proj_len_w",
        "proj_len_b",
    ]
    x = np.ascontiguousarray(inputs["x"], dtype=np.float32)
    rep = {k: np.ascontiguousarray(inputs[k], dtype=np.float32) for k in rep_keys}
    in_maps = [
        {"x": x[c * BL : (c + 1) * BL], **rep} for c in range(N_CORES)
    ]
    res = run_bass_kernel_spmd(
        nc, in_maps, core_ids=list(range(N_CORES)), trace=trace
    )
    full = np.concatenate([res.results[c]["out"] for c in range(N_CORES)], axis=0)
    return full, res


def kernel(**inputs):
    full, _ = run(inputs, trace=False)
    return full
